# revision 55
# baseline (speedup 1.0000x reference)
"""Two-layer GAT on 8 Trainium2 NeuronCores.

Strategy (dst-sharded, node-major fp16 table):
 - Nodes are degree-sorted into 128-node blocks dealt round-robin to the 8
   cores; every core runs an identical static schedule of R rounds with
   shared per-round max in-degrees (DA/DB per int16-index bank).
 - Per layer, each core computes its shard of a node table
   [row: h(64) | asrc | adst | pad -> 128 fp16 = 256B], an AllGather
   replicates the full table, then blocks of 7 rounds gather the 256B rows
   of the in-edge sources (2 dma_gather calls per block, int16-banked).
 - Per round: t = exp(leaky_relu(asrc_src + adst_dst) - 5) on ACT with
   accum_out giving the softmax denominator; DVE prescales M = t*h in fp16
   2x mode (t duplicated in packed pairs); PE accumulates sum_d M_d via
   identity matmuls with the identity kept loaded (ldweights elided).
 - Softmax max-subtraction is replaced by a constant -5 shift (exact under
   exp-normalization); padded slots point at a dummy table row with
   asrc = -1000 so t == 0 exactly.
"""
import numpy as np

_CACHE = {}
_LAST = {}

BK = 2  # rounds per gather block


def _host_prep(x, edge_index, cfg):
    N, C, R = cfg["N"], 8, cfg["R"]
    NPC = R * 128            # rows per core shard
    NTOT = C * NPC
    baseA, baseB, span = cfg["baseA"], cfg["baseB"], cfg["span"]
    A_hi = min(NTOT - 1, baseA + span)
    B_lo = max(0, baseB - span)
    assert A_hi >= B_lo - 1

    src = np.asarray(edge_index[0], dtype=np.int64)
    dst = np.asarray(edge_index[1], dtype=np.int64)
    E = src.shape[0]

    deg = np.bincount(dst, minlength=N)
    order = np.argsort(-deg, kind="stable")
    all_nodes = np.concatenate([order, np.full(NTOT - N, -1, dtype=np.int64)])

    m = np.arange(NTOT)
    b = m // 128
    p = m % 128
    rnd = b // C
    core = b % C
    # chunk-major table layout: [chunk][core][round-within-chunk][p] so each
    # chunked AllGather writes a contiguous table region
    agch = np.asarray(cfg["agch"], dtype=np.int64)
    ch = np.searchsorted(agch, rnd, side="right") - 1
    c0 = agch[ch]
    csz = agch[ch + 1] - c0
    row_of_listpos = (1024 * c0 + csz * 128 * core + 128 * (rnd - c0) + p)

    # zone-preserving re-deal: group nodes with similar (deg, forcedA) into
    # the same round while keeping every node inside its bank zone (so the
    # forced counts stay frozen). Shrinks sum of per-round bank maxes ~20%.
    row1 = np.full(N, -1, dtype=np.int64)
    real1 = all_nodes >= 0
    row1[all_nodes[real1]] = row_of_listpos[real1]
    zsrc = np.where(row1[src] < B_lo, 0, np.where(row1[src] > A_hi, 2, 1))
    fA_n = np.bincount(dst[zsrc == 0], minlength=N)
    fB_n = np.bincount(dst[zsrc == 2], minlength=N)
    zone_of_listpos = np.where(row_of_listpos < B_lo, 0,
                               np.where(row_of_listpos > A_hi, 2, 1))
    zone_of_node = np.zeros(N, dtype=np.int64)
    zone_of_node[all_nodes[real1]] = zone_of_listpos[real1]
    new_nodes = np.full(NTOT, -1, dtype=np.int64)
    for z in range(3):
        nz = np.where(zone_of_node == z)[0]
        nz_sorted = nz[np.lexsort((-fA_n[nz], -deg[nz]))]
        pos = np.where(zone_of_listpos == z)[0]
        pos_sorted = pos[np.argsort(rnd[pos], kind="stable")]
        new_nodes[pos_sorted[:len(nz_sorted)]] = nz_sorted
    all_nodes = new_nodes

    # bank holes + dummy rows must hold pad nodes
    special_rows = {baseA - 1, baseA, baseB - 1, baseB}
    row_to_listpos = np.empty(NTOT, dtype=np.int64)
    row_to_listpos[row_of_listpos] = m
    pad_positions = [i for i in range(NTOT - 1, -1, -1) if all_nodes[i] < 0]
    pi = 0
    for r in special_rows:
        lp = row_to_listpos[r]
        if all_nodes[lp] >= 0:
            while pi < len(pad_positions):
                q = pad_positions[pi]; pi += 1
                if row_of_listpos[q] not in special_rows and all_nodes[q] < 0:
                    all_nodes[lp], all_nodes[q] = all_nodes[q], all_nodes[lp]
                    break

    node_at_listpos = all_nodes
    row_of_node = np.full(N, -1, dtype=np.int64)
    real = node_at_listpos >= 0
    row_of_node[node_at_listpos[real]] = row_of_listpos[real]

    sr = row_of_node[src]
    dr = row_of_node[dst]

    # bank per edge (0=A, 1=B); per-round thresholds TA/TB chosen so that
    # DA+DB ~ max(max deg, max forcedA + max forcedB) (optimal 2-bank split)
    canA = sr <= A_hi
    canB = sr >= B_lo
    forcedA = canA & ~canB
    flex = canA & canB
    nA0 = np.bincount(dr[forcedA], minlength=NTOT)
    nf = np.bincount(dr[flex], minlength=NTOT)
    degr = np.bincount(dr, minlength=NTOT)
    nB0 = degr - nA0 - nf
    rnd_of_row = np.empty(NTOT, dtype=np.int64)
    rnd_of_row[row_of_listpos] = rnd
    mfA = np.zeros(R, dtype=np.int64)
    mfB = np.zeros(R, dtype=np.int64)
    md = np.zeros(R, dtype=np.int64)
    np.maximum.at(mfA, rnd_of_row, nA0)
    np.maximum.at(mfB, rnd_of_row, nB0)
    np.maximum.at(md, rnd_of_row, degr)
    TA = np.maximum(mfA, (md + mfA - mfB + 1) // 2)
    TB = np.maximum(mfB, md - TA)
    lo = np.maximum(nA0, degr - TB[rnd_of_row])
    hi = np.minimum(TA[rnd_of_row], nA0 + nf)
    assert np.all(lo <= hi)
    cntA = np.clip((degr + 1) // 2, lo, hi)
    o = np.argsort(dr[flex], kind="stable")
    flex_idx = np.nonzero(flex)[0][o]
    grp = dr[flex_idx]
    uniq, first = np.unique(grp, return_index=True)
    fr = np.arange(len(grp)) - first[np.searchsorted(uniq, grp)]
    bank = np.ones(E, dtype=np.int8)
    bank[forcedA] = 0
    bank[flex_idx] = (fr >= (cntA[grp] - nA0[grp])).astype(np.int8)
    cntB = degr - cntA

    rnd_of_node_row = np.empty(NTOT, dtype=np.int64)
    rnd_of_node_row[row_of_listpos] = rnd
    DA = np.zeros(R, dtype=np.int64)
    DB = np.zeros(R, dtype=np.int64)
    np.maximum.at(DA, rnd_of_node_row, cntA)
    np.maximum.at(DB, rnd_of_node_row, cntB)
    DA[(DA + DB) == 0] = 1

    # slot position within (dst, bank); negative gather indices first so the
    # final slot of each call is non-negative (HW drops trailing negatives)
    idxval = np.where(bank == 0, sr - baseA, sr - baseB)
    nonneg = (idxval >= 0).astype(np.int8)
    o2 = np.lexsort((nonneg, bank, dr))
    grp2 = dr[o2] * 2 + bank[o2]
    uniq2, first2 = np.unique(grp2, return_index=True)
    dpos = np.arange(E) - first2[np.searchsorted(uniq2, grp2)]
    d_of_edge = np.empty(E, dtype=np.int64)
    d_of_edge[o2] = dpos

    # guard: if the node at partition 127 of any core fills column dq-1 with a
    # negative idx, the call would end on a trailing negative -> add a pad col
    cnt_nonneg_A = np.bincount(dr[(bank == 0) & (idxval >= 0)], minlength=NTOT)
    cnt_nonneg_B = np.bincount(dr[(bank == 1) & (idxval >= 0)], minlength=NTOT)
    p_of_row = np.empty(NTOT, dtype=np.int64)
    p_of_row[row_of_listpos] = p
    is127 = p_of_row == 127
    fullnegA = is127 & (cntA == DA[rnd_of_node_row]) & (cntA > 0) & (cnt_nonneg_A == 0)
    fullnegB = is127 & (cntB == DB[rnd_of_node_row]) & (cntB > 0) & (cnt_nonneg_B == 0)
    for rr in np.unique(rnd_of_node_row[fullnegA]):
        DA[rr] += 1
    for rr in np.unique(rnd_of_node_row[fullnegB]):
        DB[rr] += 1

    offA = np.concatenate([[0], np.cumsum(DA)])
    offB = np.concatenate([[0], np.cumsum(DB)])
    SA, SB = int(offA[-1]), int(offB[-1])

    idxA = np.zeros((C, SA * 128), dtype=np.int32)
    idxB = np.zeros((C, SB * 128), dtype=np.int32)
    core_of_row = np.empty(NTOT, dtype=np.int64)
    core_of_row[row_of_listpos] = core
    e_core = core_of_row[dr]
    e_rnd = rnd_of_row[dr]
    e_p = p_of_row[dr]
    isA = bank == 0
    slotA = (offA[e_rnd[isA]] + d_of_edge[isA]) * 128 + e_p[isA]
    idxA[e_core[isA], slotA] = sr[isA] - baseA
    isB = ~isA
    slotB = (offB[e_rnd[isB]] + d_of_edge[isB]) * 128 + e_p[isB]
    idxB[e_core[isB], slotB] = sr[isB] - baseB
    assert idxA.min() >= -32768 and idxA.max() <= 32766
    assert idxB.min() >= -32768 and idxB.max() <= 32766
    assert not np.any(idxA == -1) and not np.any(idxB == -1)

    def wrap(a):  # [C, S*128] -> [C, 128, S*8] int16 (16-wrap, replicated x8)
        Cn, tot = a.shape
        w = a.reshape(Cn, tot // 16, 16).transpose(0, 2, 1)
        return np.ascontiguousarray(np.tile(w, (1, 8, 1))).astype(np.int16)

    xT = np.zeros((C, x.shape[1], NPC), dtype=np.float16)
    xf = np.asarray(x, dtype=np.float32)
    shardpos = 128 * rnd + p
    for k in range(C):
        sel = (core == k) & real
        cols = shardpos[sel]
        xT[k][:, cols] = xf[node_at_listpos[sel]].T.astype(np.float16)

    # concat position of each node in the per-core round-major output stack
    out_pos = np.full(N, 0, dtype=np.int64)
    out_pos[node_at_listpos[real]] = (NPC * core + shardpos)[real]

    return dict(
        idxA=wrap(idxA), idxB=wrap(idxB), xT=xT,
        DA=[int(v) for v in DA], DB=[int(v) for v in DB],
        SA=SA, SB=SB, row_of_node=row_of_node, out_pos=out_pos,
    )


def _build(cfg, DA, DB, SA, SB):
    import sys
    if "/opt/trn_rl_repo" not in sys.path:
        sys.path.insert(0, "/opt/trn_rl_repo")
    import concourse.mybir as mybir
    import concourse.tile as tile
    from concourse import bacc
    from concourse.masks import make_identity

    f32 = mybir.dt.float32
    f16 = mybir.dt.float16
    R = cfg["R"]
    F, HD = cfg["F"], cfg["H"]
    NPC = R * 128
    NTOT = 8 * NPC
    baseA, baseB = cfg["baseA"], cfg["baseB"]
    AF = HD + 3  # h | ones | asrc | adst

    offA = [0]
    for d in DA:
        offA.append(offA[-1] + d)
    offB = [0]
    for d in DB:
        offB.append(offB[-1] + d)

    CBUD = 44  # max gather columns (A+B) per block
    agset = set(cfg["agch"])
    blocks = []
    s = 0
    while s < R:
        e = s + 1
        while (e < R and e not in agset
               and (offA[e + 1] - offA[s]) + (offB[e + 1] - offB[s]) <= CBUD
               and e - s < 3):
            e += 1
        blocks.append((s, e))
        s = e
    DBLKA = max(offA[e] - offA[s] for s, e in blocks)
    DBLKB = max(offB[e] - offB[s] for s, e in blocks)
    GCAP = max(offA[e] - offA[s] + offB[e] - offB[s] for s, e in blocks)
    BMAX = max(e - s for s, e in blocks)
    DMAXB = {"a": max(max(DA), 1), "b": max(max(DB), 1)}

    nc = bacc.Bacc("TRN2", target_bir_lowering=False, debug=False, num_devices=8,
                   num_swdge_queues=4)
    xT_t = nc.dram_tensor("xT", [F, NPC], f16, kind="ExternalInput")
    iA_t = nc.dram_tensor("idxA", [128, SA * 8], mybir.dt.int16, kind="ExternalInput")
    iB_t = nc.dram_tensor("idxB", [128, SB * 8], mybir.dt.int16, kind="ExternalInput")
    aug1_t = nc.dram_tensor("aug1", [F, AF], f16, kind="ExternalInput")
    aug2_t = nc.dram_tensor("aug2", [HD, AF], f16, kind="ExternalInput")
    bb1_t = nc.dram_tensor("bb1", [128, HD], f16, kind="ExternalInput")
    bb2_t = nc.dram_tensor("bb2", [128, HD], f32, kind="ExternalInput")
    out_t = nc.dram_tensor("out", [NPC, HD], f32, kind="ExternalOutput")

    shard1 = nc.dram_tensor("shard1", [NPC, 128], f16, kind="Internal")
    shard2 = nc.dram_tensor("shard2", [NPC, 128], f16, kind="Internal")
    table1 = nc.dram_tensor("table1", [NTOT, 128], f16, kind="Internal", addr_space="Shared")
    table2 = nc.dram_tensor("table2", [NTOT, 128], f16, kind="Internal", addr_space="Shared")

    RG = [[0, 1, 2, 3, 4, 5, 6, 7]]

    with tile.TileContext(nc) as tc:
        with tc.tile_pool(name="const", bufs=1) as cp, \
             tc.tile_pool(name="gpool", bufs=8) as gp, \
             tc.tile_pool(name="mpool", bufs=4) as mp, \
             tc.tile_pool(name="spool", bufs=3) as sp, \
             tc.tile_pool(name="psA", bufs=1, space="PSUM") as psA, \
             tc.tile_pool(name="psB", bufs=6, space="PSUM") as psB, \
             tc.tile_pool(name="psT", bufs=1, space="PSUM") as psT:

            ident = cp.tile([128, 128], f16)
            make_identity(nc, ident[:])

            aug1_sb = cp.tile([F, AF], f16, tag="aug1")
            nc.sync.dma_start(out=aug1_sb[:], in_=aug1_t.ap()[:, :])
            aug2_sb = cp.tile([HD, AF], f16, tag="aug2")
            nc.sync.dma_start(out=aug2_sb[:], in_=aug2_t.ap()[:, :])
            bb1_sb = cp.tile([128, HD], f16, tag="bb1")
            nc.sync.dma_start(out=bb1_sb[:], in_=bb1_t.ap()[:, :])
            bb2_sb = cp.tile([128, HD], f32, tag="bb2")
            nc.sync.dma_start(out=bb2_sb[:], in_=bb2_t.ap()[:, :])

            dumrow = cp.tile([1, 128], f16)
            nc.vector.memset(dumrow[:], 0.0)
            nc.vector.memset(dumrow[:, HD + 1:HD + 2], -1000.0)

            negbias = cp.tile([128, 1], f32, tag="negbias")
            nc.vector.memset(negbias[:], -5.0)

            cb0 = cp.tile([128, R], f32, tag="cb0")
            cb1 = cp.tile([128, R], f32, tag="cb1")
            cbt = [cb0, cb1]

            iA_sb = cp.tile([128, SA * 8], mybir.dt.int16)
            nc.sync.dma_start(out=iA_sb[:], in_=iA_t.ap()[:, :])
            iB_sb = cp.tile([128, SB * 8], mybir.dt.int16)
            nc.sync.dma_start(out=iB_sb[:], in_=iB_t.ap()[:, :])

            adst_sb0 = cp.tile([128, R], f32, tag="adst0")
            adst_sb1 = cp.tile([128, R], f32, tag="adst1")
            adst_sb = [adst_sb0, adst_sb1]

            SLAB = 7

            def phase_A1():
                for s0 in range(0, R, SLAB):
                    s1 = min(s0 + SLAB, R)
                    slab = sp.tile([F, 128 * SLAB], f16, tag="paslab")
                    nc.sync.dma_start(out=slab[:, 0:128 * (s1 - s0)],
                                      in_=xT_t.ap()[:, 128 * s0:128 * s1])
                    cslab = sp.tile([128, SLAB, AF], f16, tag="pacslab")
                    for t in range(s0, s1):
                        xc = slab[:, 128 * (t - s0):128 * (t - s0 + 1)]
                        hp = psA.tile([128, AF], f32, tag="paps")
                        nc.tensor.matmul(out=hp[:], lhsT=xc, rhs=aug1_sb[:],
                                         start=True, stop=True)
                        nc.scalar.copy(out=cslab[:, t - s0, :], in_=hp[:])
                        nc.vector.memset(cslab[:, t - s0, HD:HD + 1], 1.0)
                        nc.vector.tensor_copy(out=adst_sb[0][:, t:t + 1],
                                              in_=hp[:, HD + 2:HD + 3])
                    nc.sync.dma_start(
                        out=shard1.ap()[128 * s0:128 * s1, 0:AF].rearrange(
                            "(n p) f -> p n f", p=128),
                        in_=cslab[:, 0:s1 - s0, :])
                    for t in range(s0, s1):
                        if t + 1 in AGCH:
                            allgather_chunk(shard1, table1,
                                            AGCH.index(t + 1) - 1)

            AGCH = cfg["agch"]

            def allgather_chunk(shard, table, k):
                lo, hi = AGCH[k] * 128, AGCH[k + 1] * 128
                nc.gpsimd.collective_compute(
                    "AllGather", mybir.AluOpType.bypass, RG,
                    ins=[shard.ap()[lo:hi, :]],
                    outs=[table.ap()[lo * 8:hi * 8, :]])

            def patch_dummy(table):
                nc.sync.dma_start(out=table.ap()[baseA:baseA + 1, :], in_=dumrow[:])
                nc.sync.dma_start(out=table.ap()[baseB:baseB + 1, :], in_=dumrow[:])

            qctr = [0]

            def pick_q(cols):
                q = qctr[0] % 4
                qctr[0] += 1
                return q

            def phase_B(layer, table, adst):
                final = layer == 1

                def stage2_block(b0, b1, pos):
                    nb = b1 - b0
                    if final:
                        oslab = sp.tile([128, BMAX, HD], f32, tag="oslab")
                    else:
                        c2slab = sp.tile([128, BMAX, AF], f16, tag="c2slab")
                    for r in range(b0, b1):
                        po = pos[r - b0]
                        den2 = mp.tile([128, 1], f32, tag="den2")
                        nc.vector.tensor_scalar_max(out=den2[:],
                                                    in0=po[:, HD:HD + 1],
                                                    scalar1=1e-30)
                        rd = mp.tile([128, 1], f32, tag="rd")
                        nc.vector.reciprocal(out=rd[:], in_=den2[:])
                        if final:
                            hf = oslab[:, r - b0, :]
                            nc.scalar.activation(
                                out=hf, in_=po[:, 0:HD],
                                func=mybir.ActivationFunctionType.Copy,
                                scale=rd[:])
                            nc.vector.tensor_tensor(out=hf, in0=hf,
                                                    in1=bb2_sb[:],
                                                    op=mybir.AluOpType.add)
                        else:
                            h2 = sp.tile([128, HD], f16, tag="h2")
                            nc.scalar.activation(
                                out=h2[:], in_=po[:, 0:HD],
                                func=mybir.ActivationFunctionType.Copy,
                                scale=rd[:])
                            nc.vector.tensor_tensor(out=h2[:], in0=h2[:],
                                                    in1=bb1_sb[:],
                                                    op=mybir.AluOpType.add)
                            nc.vector.tensor_scalar_max(out=h2[:], in0=h2[:],
                                                        scalar1=0.0)
                            htp = psT.tile([HD, 128], f16, tag="htp")
                            nc.tensor.transpose(out=htp[:], in_=h2[:],
                                                identity=ident[:])
                            hT = sp.tile([HD, 128], f16, tag="hT")
                            nc.scalar.copy(out=hT[:], in_=htp[:])
                            hp2 = psA.tile([128, AF], f32, tag="paps")
                            nc.tensor.matmul(out=hp2[:], lhsT=hT[:],
                                             rhs=aug2_sb[:], start=True,
                                             stop=True)
                            nc.scalar.copy(out=c2slab[:, r - b0, :],
                                           in_=hp2[:])
                            nc.vector.memset(c2slab[:, r - b0, HD:HD + 1], 1.0)
                            nc.vector.tensor_copy(out=adst_sb[1][:, r:r + 1],
                                                  in_=hp2[:, HD + 2:HD + 3])
                    if final:
                        nc.sync.dma_start(
                            out=out_t.ap()[128 * b0:128 * b1, :].rearrange(
                                "(n p) f -> p n f", p=128),
                            in_=oslab[:, 0:nb, :])
                    else:
                        nc.sync.dma_start(
                            out=shard2.ap()[128 * b0:128 * b1, 0:AF].rearrange(
                                "(n p) f -> p n f", p=128),
                            in_=c2slab[:, 0:nb, :])
                        if b1 in AGCH:
                            allgather_chunk(shard2, table2,
                                            AGCH.index(b1) - 1)

                pend = None
                for (b0, b1) in blocks:
                    aw = offA[b1] - offA[b0]
                    bw = offB[b1] - offB[b0]
                    w = aw + bw
                    G = gp.tile([128, GCAP, 128], f16, tag="G")
                    if aw:
                        nc.gpsimd.dma_gather(
                            out_ap=G[:, 0:aw, :], in_ap=table.ap()[baseA:, :],
                            idxs_ap=iA_sb[:, offA[b0] * 8:offA[b1] * 8],
                            num_idxs=128 * aw, num_idxs_reg=128 * aw,
                            elem_size=128, single_packet=False,
                            queue_num=pick_q(aw))
                    if bw:
                        nc.gpsimd.dma_gather(
                            out_ap=G[:, aw:w, :], in_ap=table.ap()[baseB:, :],
                            idxs_ap=iB_sb[:, offB[b0] * 8:offB[b1] * 8],
                            num_idxs=128 * bw, num_idxs_reg=128 * bw,
                            elem_size=128, single_packet=False,
                            queue_num=pick_q(bw))

                    e1 = mp.tile([128, GCAP, 1], f32, tag="e1")
                    nc.scalar.activation(
                        out=e1[:, 0:w, :], in_=G[:, 0:w, HD + 1:HD + 2],
                        func=mybir.ActivationFunctionType.Exp,
                        bias=negbias[:], scale=1.0)
                    e2 = mp.tile([128, GCAP, 1], f32, tag="e2")
                    nc.scalar.activation(
                        out=e2[:, 0:w, :], in_=G[:, 0:w, HD + 1:HD + 2],
                        func=mybir.ActivationFunctionType.Exp,
                        bias=negbias[:], scale=cfg["slope"])

                    pos = []
                    for r in range(b0, b1):
                        da, db = DA[r], DB[r]
                        a0 = offA[r] - offA[b0]
                        bq0 = aw + offB[r] - offB[b0]
                        po = psB.tile([128, 128], f32, tag="po")
                        pos.append(po)
                        first = True
                        for (g0, dq, btag) in ((a0, da, "a"), (bq0, db, "b")):
                            if dq == 0:
                                continue
                            tt = mp.tile([128, DMAXB[btag], 1], f16,
                                         tag="tt" + btag)
                            nc.vector.scalar_tensor_tensor(
                                out=tt[:, 0:dq, :],
                                in0=e2[:, g0:g0 + dq, :],
                                scalar=adst[:, r:r + 1],
                                in1=e1[:, g0:g0 + dq, :],
                                op0=mybir.AluOpType.mult,
                                op1=mybir.AluOpType.max)
                            M = mp.tile([128, DMAXB[btag], HD + 1], f16,
                                        tag="M" + btag)
                            nc.vector.tensor_tensor(
                                out=M[:, 0:dq, :],
                                in0=G[:, g0:g0 + dq, 0:HD + 1],
                                in1=tt[:, 0:dq, :].to_broadcast(
                                    [128, dq, HD + 1]),
                                op=mybir.AluOpType.mult)
                            last_bank = (btag == "b") or db == 0
                            for d in range(dq):
                                mm = nc.tensor.matmul(
                                    out=po[:, 0:HD + 1], lhsT=ident[:],
                                    rhs=M[:, d, :],
                                    start=first, stop=last_bank and d == dq - 1)
                                if not first:
                                    mm.ldweights = False
                                first = False
                    if pend is not None:
                        stage2_block(*pend)
                    pend = (b0, b1, pos)
                stage2_block(*pend)

            def make_biases(layer):
                # c_j = exp((slope-1)*adst_j); exp(adst_j) factors out of the
                # segment softmax so only this ratio term remains per-round
                nc.scalar.activation(
                    out=cbt[layer][:], in_=adst_sb[layer][:],
                    func=mybir.ActivationFunctionType.Exp,
                    bias=0.0, scale=cfg["slope"] - 1.0)

            phase_A1()
            patch_dummy(table1)
            make_biases(0)
            phase_B(0, table1, cbt[0])
            patch_dummy(table2)
            make_biases(1)
            phase_B(1, table2, cbt[1])

    nc.compile()
    return nc


def _agch(R):
    if R < 8:
        return [0, R]
    return [0, (R * 6) // 7, R]


def _make_cfg(N, F, H):
    if N >= 32768:
        return dict(N=N, R=98, baseA=32768, baseB=67585, span=32766,
                    F=F, H=H, slope=0.2, agch=_agch(98))
    NTOT = max(2048, ((N + 128 + 1023) // 1024) * 1024)
    R = NTOT // 1024
    return dict(N=N, R=R, baseA=NTOT // 4, baseB=(3 * NTOT) // 4,
                span=min(32766, (5 * NTOT) // 8), F=F, H=H, slope=0.2,
                agch=_agch(R))


def kernel(x, edge_index, W1, a1_src, a1_dst, b1, W2, a2_src, a2_dst, b2):
    import sys
    if "/opt/trn_rl_repo" not in sys.path:
        sys.path.insert(0, "/opt/trn_rl_repo")
    from concourse import bass_utils

    x = np.asarray(x)
    cfg = _make_cfg(x.shape[0], x.shape[1], np.asarray(W1).shape[1])
    prep = _host_prep(x, edge_index, cfg)
    key = (cfg["N"], cfg["R"], prep["SA"], prep["SB"],
           tuple(prep["DA"]), tuple(prep["DB"]))
    if key not in _CACHE:
        _CACHE[key] = _build(cfg, prep["DA"], prep["DB"], prep["SA"], prep["SB"])
    nc = _CACHE[key]

    W1f = np.asarray(W1, dtype=np.float32)
    W2f = np.asarray(W2, dtype=np.float32)
    aug1 = np.concatenate(
        [W1f, np.zeros((W1f.shape[0], 1), dtype=np.float32),
         (W1f @ np.asarray(a1_src, dtype=np.float32))[:, None],
         (W1f @ np.asarray(a1_dst, dtype=np.float32))[:, None]],
        axis=1).astype(np.float16)
    aug2 = np.concatenate(
        [W2f, np.zeros((W2f.shape[0], 1), dtype=np.float32),
         (W2f @ np.asarray(a2_src, dtype=np.float32))[:, None],
         (W2f @ np.asarray(a2_dst, dtype=np.float32))[:, None]],
        axis=1).astype(np.float16)
    bb1 = np.tile(np.asarray(b1, dtype=np.float16)[None, :], (128, 1))
    bb2 = np.tile(np.asarray(b2, dtype=np.float32)[None, :], (128, 1))

    in_maps = []
    for k in range(8):
        in_maps.append({
            "xT": prep["xT"][k], "idxA": prep["idxA"][k], "idxB": prep["idxB"][k],
            "aug1": aug1, "aug2": aug2, "bb1": bb1, "bb2": bb2,
        })
    _LAST["nc"] = nc
    _LAST["in_maps"] = in_maps
    res = bass_utils.run_bass_kernel_spmd(nc, in_maps, core_ids=list(range(8)))
    shards = np.concatenate([res.results[k]["out"] for k in range(8)], axis=0)
    return shards[prep["out_pos"]].astype(np.float32)



# revision 56
# speedup vs baseline: 1.0478x; 1.0478x over previous
"""Two-layer GAT on 8 Trainium2 NeuronCores.

Strategy (dst-sharded, node-major fp16 table):
 - Nodes are degree-sorted into 128-node blocks dealt round-robin to the 8
   cores; every core runs an identical static schedule of R rounds with
   shared per-round max in-degrees (DA/DB per int16-index bank).
 - Per layer, each core computes its shard of a node table
   [row: h(64) | asrc | adst | pad -> 128 fp16 = 256B], an AllGather
   replicates the full table, then blocks of 7 rounds gather the 256B rows
   of the in-edge sources (2 dma_gather calls per block, int16-banked).
 - Per round: t = exp(leaky_relu(asrc_src + adst_dst) - 5) on ACT with
   accum_out giving the softmax denominator; DVE prescales M = t*h in fp16
   2x mode (t duplicated in packed pairs); PE accumulates sum_d M_d via
   identity matmuls with the identity kept loaded (ldweights elided).
 - Softmax max-subtraction is replaced by a constant -5 shift (exact under
   exp-normalization); padded slots point at a dummy table row with
   asrc = -1000 so t == 0 exactly.
"""
import numpy as np

_CACHE = {}
_LAST = {}

BK = 2  # rounds per gather block


def _host_prep(x, edge_index, cfg):
    N, C, R = cfg["N"], 8, cfg["R"]
    NPC = R * 128            # rows per core shard
    NTOT = C * NPC
    baseA, baseB, span = cfg["baseA"], cfg["baseB"], cfg["span"]
    A_hi = min(NTOT - 1, baseA + span)
    B_lo = max(0, baseB - span)
    assert A_hi >= B_lo - 1

    src = np.asarray(edge_index[0], dtype=np.int64)
    dst = np.asarray(edge_index[1], dtype=np.int64)
    E = src.shape[0]

    deg = np.bincount(dst, minlength=N)
    order = np.argsort(-deg, kind="stable")
    all_nodes = np.concatenate([order, np.full(NTOT - N, -1, dtype=np.int64)])

    m = np.arange(NTOT)
    b = m // 128
    p = m % 128
    rnd = b // C
    core = b % C
    # chunk-major table layout: [chunk][core][round-within-chunk][p] so each
    # chunked AllGather writes a contiguous table region
    agch = np.asarray(cfg["agch"], dtype=np.int64)
    ch = np.searchsorted(agch, rnd, side="right") - 1
    c0 = agch[ch]
    csz = agch[ch + 1] - c0
    row_of_listpos = (1024 * c0 + csz * 128 * core + 128 * (rnd - c0) + p)

    # zone-preserving re-deal: group nodes with similar (deg, forcedA) into
    # the same round while keeping every node inside its bank zone (so the
    # forced counts stay frozen). Shrinks sum of per-round bank maxes ~20%.
    row1 = np.full(N, -1, dtype=np.int64)
    real1 = all_nodes >= 0
    row1[all_nodes[real1]] = row_of_listpos[real1]
    zsrc = np.where(row1[src] < B_lo, 0, np.where(row1[src] > A_hi, 2, 1))
    fA_n = np.bincount(dst[zsrc == 0], minlength=N)
    fB_n = np.bincount(dst[zsrc == 2], minlength=N)
    zone_of_listpos = np.where(row_of_listpos < B_lo, 0,
                               np.where(row_of_listpos > A_hi, 2, 1))
    zone_of_node = np.zeros(N, dtype=np.int64)
    zone_of_node[all_nodes[real1]] = zone_of_listpos[real1]
    new_nodes = np.full(NTOT, -1, dtype=np.int64)
    for z in range(3):
        nz = np.where(zone_of_node == z)[0]
        nz_sorted = nz[np.lexsort((-fA_n[nz], -deg[nz]))]
        pos = np.where(zone_of_listpos == z)[0]
        pos_sorted = pos[np.argsort(rnd[pos], kind="stable")]
        new_nodes[pos_sorted[:len(nz_sorted)]] = nz_sorted
    all_nodes = new_nodes

    # bank holes + dummy rows must hold pad nodes
    special_rows = {baseA - 1, baseA, baseB - 1, baseB}
    row_to_listpos = np.empty(NTOT, dtype=np.int64)
    row_to_listpos[row_of_listpos] = m
    pad_positions = [i for i in range(NTOT - 1, -1, -1) if all_nodes[i] < 0]
    pi = 0
    for r in special_rows:
        lp = row_to_listpos[r]
        if all_nodes[lp] >= 0:
            while pi < len(pad_positions):
                q = pad_positions[pi]; pi += 1
                if row_of_listpos[q] not in special_rows and all_nodes[q] < 0:
                    all_nodes[lp], all_nodes[q] = all_nodes[q], all_nodes[lp]
                    break

    node_at_listpos = all_nodes
    row_of_node = np.full(N, -1, dtype=np.int64)
    real = node_at_listpos >= 0
    row_of_node[node_at_listpos[real]] = row_of_listpos[real]

    sr = row_of_node[src]
    dr = row_of_node[dst]

    # bank per edge (0=A, 1=B); per-round thresholds TA/TB chosen so that
    # DA+DB ~ max(max deg, max forcedA + max forcedB) (optimal 2-bank split)
    canA = sr <= A_hi
    canB = sr >= B_lo
    forcedA = canA & ~canB
    flex = canA & canB
    nA0 = np.bincount(dr[forcedA], minlength=NTOT)
    nf = np.bincount(dr[flex], minlength=NTOT)
    degr = np.bincount(dr, minlength=NTOT)
    nB0 = degr - nA0 - nf
    rnd_of_row = np.empty(NTOT, dtype=np.int64)
    rnd_of_row[row_of_listpos] = rnd
    mfA = np.zeros(R, dtype=np.int64)
    mfB = np.zeros(R, dtype=np.int64)
    md = np.zeros(R, dtype=np.int64)
    np.maximum.at(mfA, rnd_of_row, nA0)
    np.maximum.at(mfB, rnd_of_row, nB0)
    np.maximum.at(md, rnd_of_row, degr)
    TA = np.maximum(mfA, (md + mfA - mfB + 1) // 2)
    TB = np.maximum(mfB, md - TA)
    lo = np.maximum(nA0, degr - TB[rnd_of_row])
    hi = np.minimum(TA[rnd_of_row], nA0 + nf)
    assert np.all(lo <= hi)
    cntA = np.clip((degr + 1) // 2, lo, hi)
    o = np.argsort(dr[flex], kind="stable")
    flex_idx = np.nonzero(flex)[0][o]
    grp = dr[flex_idx]
    uniq, first = np.unique(grp, return_index=True)
    fr = np.arange(len(grp)) - first[np.searchsorted(uniq, grp)]
    bank = np.ones(E, dtype=np.int8)
    bank[forcedA] = 0
    bank[flex_idx] = (fr >= (cntA[grp] - nA0[grp])).astype(np.int8)
    cntB = degr - cntA

    rnd_of_node_row = np.empty(NTOT, dtype=np.int64)
    rnd_of_node_row[row_of_listpos] = rnd
    DA = np.zeros(R, dtype=np.int64)
    DB = np.zeros(R, dtype=np.int64)
    np.maximum.at(DA, rnd_of_node_row, cntA)
    np.maximum.at(DB, rnd_of_node_row, cntB)
    DA[(DA + DB) == 0] = 1

    # slot position within (dst, bank); negative gather indices first so the
    # final slot of each call is non-negative (HW drops trailing negatives)
    idxval = np.where(bank == 0, sr - baseA, sr - baseB)
    nonneg = (idxval >= 0).astype(np.int8)
    o2 = np.lexsort((nonneg, bank, dr))
    grp2 = dr[o2] * 2 + bank[o2]
    uniq2, first2 = np.unique(grp2, return_index=True)
    dpos = np.arange(E) - first2[np.searchsorted(uniq2, grp2)]
    d_of_edge = np.empty(E, dtype=np.int64)
    d_of_edge[o2] = dpos

    # guard: if the node at partition 127 of any core fills column dq-1 with a
    # negative idx, the call would end on a trailing negative -> add a pad col
    cnt_nonneg_A = np.bincount(dr[(bank == 0) & (idxval >= 0)], minlength=NTOT)
    cnt_nonneg_B = np.bincount(dr[(bank == 1) & (idxval >= 0)], minlength=NTOT)
    p_of_row = np.empty(NTOT, dtype=np.int64)
    p_of_row[row_of_listpos] = p
    is127 = p_of_row == 127
    fullnegA = is127 & (cntA == DA[rnd_of_node_row]) & (cntA > 0) & (cnt_nonneg_A == 0)
    fullnegB = is127 & (cntB == DB[rnd_of_node_row]) & (cntB > 0) & (cnt_nonneg_B == 0)
    for rr in np.unique(rnd_of_node_row[fullnegA]):
        DA[rr] += 1
    for rr in np.unique(rnd_of_node_row[fullnegB]):
        DB[rr] += 1

    offA = np.concatenate([[0], np.cumsum(DA)])
    offB = np.concatenate([[0], np.cumsum(DB)])
    SA, SB = int(offA[-1]), int(offB[-1])

    idxA = np.zeros((C, SA * 128), dtype=np.int32)
    idxB = np.zeros((C, SB * 128), dtype=np.int32)
    core_of_row = np.empty(NTOT, dtype=np.int64)
    core_of_row[row_of_listpos] = core
    e_core = core_of_row[dr]
    e_rnd = rnd_of_row[dr]
    e_p = p_of_row[dr]
    isA = bank == 0
    slotA = (offA[e_rnd[isA]] + d_of_edge[isA]) * 128 + e_p[isA]
    idxA[e_core[isA], slotA] = sr[isA] - baseA
    isB = ~isA
    slotB = (offB[e_rnd[isB]] + d_of_edge[isB]) * 128 + e_p[isB]
    idxB[e_core[isB], slotB] = sr[isB] - baseB
    assert idxA.min() >= -32768 and idxA.max() <= 32766
    assert idxB.min() >= -32768 and idxB.max() <= 32766
    assert not np.any(idxA == -1) and not np.any(idxB == -1)

    def wrap(a):  # [C, S*128] -> [C, 128, S*8] int16 (16-wrap, replicated x8)
        Cn, tot = a.shape
        w = a.reshape(Cn, tot // 16, 16).transpose(0, 2, 1)
        return np.ascontiguousarray(np.tile(w, (1, 8, 1))).astype(np.int16)

    xT = np.zeros((C, x.shape[1], NPC), dtype=np.float16)
    xf = np.asarray(x, dtype=np.float32)
    shardpos = 128 * rnd + p
    for k in range(C):
        sel = (core == k) & real
        cols = shardpos[sel]
        xT[k][:, cols] = xf[node_at_listpos[sel]].T.astype(np.float16)

    # concat position of each node in the per-core round-major output stack
    out_pos = np.full(N, 0, dtype=np.int64)
    out_pos[node_at_listpos[real]] = (NPC * core + shardpos)[real]

    return dict(
        idxA=wrap(idxA), idxB=wrap(idxB), xT=xT,
        DA=[int(v) for v in DA], DB=[int(v) for v in DB],
        SA=SA, SB=SB, row_of_node=row_of_node, out_pos=out_pos,
    )


def _build(cfg, DA, DB, SA, SB):
    import sys
    if "/opt/trn_rl_repo" not in sys.path:
        sys.path.insert(0, "/opt/trn_rl_repo")
    import concourse.mybir as mybir
    import concourse.tile as tile
    from concourse import bacc
    from concourse.masks import make_identity

    f32 = mybir.dt.float32
    f16 = mybir.dt.float16
    R = cfg["R"]
    F, HD = cfg["F"], cfg["H"]
    NPC = R * 128
    NTOT = 8 * NPC
    baseA, baseB = cfg["baseA"], cfg["baseB"]
    AF = HD + 3  # h | ones | asrc | adst

    offA = [0]
    for d in DA:
        offA.append(offA[-1] + d)
    offB = [0]
    for d in DB:
        offB.append(offB[-1] + d)

    CBUD = 44  # max gather columns (A+B) per block
    agset = set(cfg["agch"])
    blocks = []
    s = 0
    while s < R:
        e = s + 1
        while (e < R and e not in agset
               and (offA[e + 1] - offA[s]) + (offB[e + 1] - offB[s]) <= CBUD
               and e - s < 2):
            e += 1
        blocks.append((s, e))
        s = e
    DBLKA = max(offA[e] - offA[s] for s, e in blocks)
    DBLKB = max(offB[e] - offB[s] for s, e in blocks)
    GCAP = max(offA[e] - offA[s] + offB[e] - offB[s] for s, e in blocks)
    BMAX = max(e - s for s, e in blocks)
    DMAXB = {"a": max(max(DA), 1), "b": max(max(DB), 1)}

    nc = bacc.Bacc("TRN2", target_bir_lowering=False, debug=False, num_devices=8,
                   num_swdge_queues=4)
    xT_t = nc.dram_tensor("xT", [F, NPC], f16, kind="ExternalInput")
    iA_t = nc.dram_tensor("idxA", [128, SA * 8], mybir.dt.int16, kind="ExternalInput")
    iB_t = nc.dram_tensor("idxB", [128, SB * 8], mybir.dt.int16, kind="ExternalInput")
    aug1_t = nc.dram_tensor("aug1", [F, AF], f16, kind="ExternalInput")
    aug2_t = nc.dram_tensor("aug2", [HD, AF], f16, kind="ExternalInput")
    bb1_t = nc.dram_tensor("bb1", [128, HD], f16, kind="ExternalInput")
    bb2_t = nc.dram_tensor("bb2", [128, HD], f32, kind="ExternalInput")
    out_t = nc.dram_tensor("out", [NPC, HD], f32, kind="ExternalOutput")

    shard1 = nc.dram_tensor("shard1", [NPC, 128], f16, kind="Internal")
    shard2 = nc.dram_tensor("shard2", [NPC, 128], f16, kind="Internal")
    table1 = nc.dram_tensor("table1", [NTOT, 128], f16, kind="Internal", addr_space="Shared")
    table2 = nc.dram_tensor("table2", [NTOT, 128], f16, kind="Internal", addr_space="Shared")

    RG = [[0, 1, 2, 3, 4, 5, 6, 7]]

    with tile.TileContext(nc) as tc:
        with tc.tile_pool(name="const", bufs=1) as cp, \
             tc.tile_pool(name="gpool", bufs=8) as gp, \
             tc.tile_pool(name="mpool", bufs=4) as mp, \
             tc.tile_pool(name="spool", bufs=3) as sp, \
             tc.tile_pool(name="psA", bufs=2, space="PSUM") as psA, \
             tc.tile_pool(name="psB", bufs=5, space="PSUM") as psB, \
             tc.tile_pool(name="psT", bufs=1, space="PSUM") as psT:

            ident = cp.tile([128, 128], f16)
            make_identity(nc, ident[:])

            aug1_sb = cp.tile([F, AF], f16, tag="aug1")
            nc.sync.dma_start(out=aug1_sb[:], in_=aug1_t.ap()[:, :])
            aug2_sb = cp.tile([HD, AF], f16, tag="aug2")
            nc.sync.dma_start(out=aug2_sb[:], in_=aug2_t.ap()[:, :])
            bb1_sb = cp.tile([128, HD], f16, tag="bb1")
            nc.sync.dma_start(out=bb1_sb[:], in_=bb1_t.ap()[:, :])
            bb2_sb = cp.tile([128, HD], f32, tag="bb2")
            nc.sync.dma_start(out=bb2_sb[:], in_=bb2_t.ap()[:, :])

            dumrow = cp.tile([1, 128], f16)
            nc.vector.memset(dumrow[:], 0.0)
            nc.vector.memset(dumrow[:, HD + 1:HD + 2], -1000.0)

            negbias = cp.tile([128, 1], f32, tag="negbias")
            nc.vector.memset(negbias[:], -5.0)

            cb0 = cp.tile([128, R], f32, tag="cb0")
            cb1 = cp.tile([128, R], f32, tag="cb1")
            cbt = [cb0, cb1]

            iA_sb = cp.tile([128, SA * 8], mybir.dt.int16)
            nc.sync.dma_start(out=iA_sb[:], in_=iA_t.ap()[:, :])
            iB_sb = cp.tile([128, SB * 8], mybir.dt.int16)
            nc.sync.dma_start(out=iB_sb[:], in_=iB_t.ap()[:, :])

            adst_sb0 = cp.tile([128, R], f32, tag="adst0")
            adst_sb1 = cp.tile([128, R], f32, tag="adst1")
            adst_sb = [adst_sb0, adst_sb1]

            SLAB = 7

            def phase_A1():
                for s0 in range(0, R, SLAB):
                    s1 = min(s0 + SLAB, R)
                    slab = sp.tile([F, 128 * SLAB], f16, tag="paslab")
                    nc.sync.dma_start(out=slab[:, 0:128 * (s1 - s0)],
                                      in_=xT_t.ap()[:, 128 * s0:128 * s1])
                    cslab = sp.tile([128, SLAB, AF], f16, tag="pacslab")
                    for t in range(s0, s1):
                        xc = slab[:, 128 * (t - s0):128 * (t - s0 + 1)]
                        hp = psA.tile([128, AF], f32, tag="paps")
                        nc.tensor.matmul(out=hp[:], lhsT=xc, rhs=aug1_sb[:],
                                         start=True, stop=True)
                        nc.scalar.copy(out=cslab[:, t - s0, :], in_=hp[:])
                        nc.vector.memset(cslab[:, t - s0, HD:HD + 1], 1.0)
                        nc.vector.tensor_copy(out=adst_sb[0][:, t:t + 1],
                                              in_=hp[:, HD + 2:HD + 3])
                    nc.sync.dma_start(
                        out=shard1.ap()[128 * s0:128 * s1, 0:AF].rearrange(
                            "(n p) f -> p n f", p=128),
                        in_=cslab[:, 0:s1 - s0, :])
                    for t in range(s0, s1):
                        if t + 1 in AGCH:
                            allgather_chunk(shard1, table1,
                                            AGCH.index(t + 1) - 1)

            AGCH = cfg["agch"]

            def allgather_chunk(shard, table, k):
                lo, hi = AGCH[k] * 128, AGCH[k + 1] * 128
                nc.gpsimd.collective_compute(
                    "AllGather", mybir.AluOpType.bypass, RG,
                    ins=[shard.ap()[lo:hi, :]],
                    outs=[table.ap()[lo * 8:hi * 8, :]])

            def patch_dummy(table):
                nc.sync.dma_start(out=table.ap()[baseA:baseA + 1, :], in_=dumrow[:])
                nc.sync.dma_start(out=table.ap()[baseB:baseB + 1, :], in_=dumrow[:])

            qctr = [0]

            def pick_q(cols):
                q = qctr[0] % 4
                qctr[0] += 1
                return q

            def phase_B(layer, table, adst):
                final = layer == 1

                def stage2_block(b0, b1, pos):
                    nb = b1 - b0
                    if final:
                        oslab = sp.tile([128, BMAX, HD], f32, tag="oslab")
                    else:
                        c2slab = sp.tile([128, BMAX, AF], f16, tag="c2slab")
                    for r in range(b0, b1):
                        po = pos[r - b0]
                        den2 = mp.tile([128, 1], f32, tag="den2")
                        nc.vector.tensor_scalar_max(out=den2[:],
                                                    in0=po[:, HD:HD + 1],
                                                    scalar1=1e-30)
                        rd = mp.tile([128, 1], f32, tag="rd")
                        nc.vector.reciprocal(out=rd[:], in_=den2[:])
                        if final:
                            hf = oslab[:, r - b0, :]
                            nc.scalar.activation(
                                out=hf, in_=po[:, 0:HD],
                                func=mybir.ActivationFunctionType.Copy,
                                scale=rd[:])
                            nc.vector.tensor_tensor(out=hf, in0=hf,
                                                    in1=bb2_sb[:],
                                                    op=mybir.AluOpType.add)
                        else:
                            h2 = sp.tile([128, HD], f16, tag="h2")
                            nc.scalar.activation(
                                out=h2[:], in_=po[:, 0:HD],
                                func=mybir.ActivationFunctionType.Copy,
                                scale=rd[:])
                            nc.vector.tensor_tensor(out=h2[:], in0=h2[:],
                                                    in1=bb1_sb[:],
                                                    op=mybir.AluOpType.add)
                            nc.vector.tensor_scalar_max(out=h2[:], in0=h2[:],
                                                        scalar1=0.0)
                            htp = psT.tile([HD, 128], f16, tag="htp")
                            nc.tensor.transpose(out=htp[:], in_=h2[:],
                                                identity=ident[:])
                            hT = sp.tile([HD, 128], f16, tag="hT")
                            nc.scalar.copy(out=hT[:], in_=htp[:])
                            hp2 = psA.tile([128, AF], f32, tag="paps")
                            nc.tensor.matmul(out=hp2[:], lhsT=hT[:],
                                             rhs=aug2_sb[:], start=True,
                                             stop=True)
                            nc.scalar.copy(out=c2slab[:, r - b0, :],
                                           in_=hp2[:])
                            nc.vector.memset(c2slab[:, r - b0, HD:HD + 1], 1.0)
                            nc.vector.tensor_copy(out=adst_sb[1][:, r:r + 1],
                                                  in_=hp2[:, HD + 2:HD + 3])
                    if final:
                        nc.sync.dma_start(
                            out=out_t.ap()[128 * b0:128 * b1, :].rearrange(
                                "(n p) f -> p n f", p=128),
                            in_=oslab[:, 0:nb, :])
                    else:
                        nc.sync.dma_start(
                            out=shard2.ap()[128 * b0:128 * b1, 0:AF].rearrange(
                                "(n p) f -> p n f", p=128),
                            in_=c2slab[:, 0:nb, :])
                        if b1 in AGCH:
                            allgather_chunk(shard2, table2,
                                            AGCH.index(b1) - 1)

                pend = None
                for (b0, b1) in blocks:
                    aw = offA[b1] - offA[b0]
                    bw = offB[b1] - offB[b0]
                    w = aw + bw
                    G = gp.tile([128, GCAP, 128], f16, tag="G")
                    if aw:
                        nc.gpsimd.dma_gather(
                            out_ap=G[:, 0:aw, :], in_ap=table.ap()[baseA:, :],
                            idxs_ap=iA_sb[:, offA[b0] * 8:offA[b1] * 8],
                            num_idxs=128 * aw, num_idxs_reg=128 * aw,
                            elem_size=128, single_packet=False,
                            queue_num=pick_q(aw))
                    if bw:
                        nc.gpsimd.dma_gather(
                            out_ap=G[:, aw:w, :], in_ap=table.ap()[baseB:, :],
                            idxs_ap=iB_sb[:, offB[b0] * 8:offB[b1] * 8],
                            num_idxs=128 * bw, num_idxs_reg=128 * bw,
                            elem_size=128, single_packet=False,
                            queue_num=pick_q(bw))

                    e1 = mp.tile([128, GCAP, 1], f32, tag="e1")
                    nc.scalar.activation(
                        out=e1[:, 0:w, :], in_=G[:, 0:w, HD + 1:HD + 2],
                        func=mybir.ActivationFunctionType.Exp,
                        bias=negbias[:], scale=1.0)
                    e2 = mp.tile([128, GCAP, 1], f32, tag="e2")
                    nc.scalar.activation(
                        out=e2[:, 0:w, :], in_=G[:, 0:w, HD + 1:HD + 2],
                        func=mybir.ActivationFunctionType.Exp,
                        bias=negbias[:], scale=cfg["slope"])

                    pos = []
                    for r in range(b0, b1):
                        da, db = DA[r], DB[r]
                        a0 = offA[r] - offA[b0]
                        bq0 = aw + offB[r] - offB[b0]
                        po = psB.tile([128, 128], f32, tag="po")
                        pos.append(po)
                        first = True
                        for (g0, dq, btag) in ((a0, da, "a"), (bq0, db, "b")):
                            if dq == 0:
                                continue
                            tt = mp.tile([128, DMAXB[btag], 1], f16,
                                         tag="tt" + btag)
                            nc.vector.scalar_tensor_tensor(
                                out=tt[:, 0:dq, :],
                                in0=e2[:, g0:g0 + dq, :],
                                scalar=adst[:, r:r + 1],
                                in1=e1[:, g0:g0 + dq, :],
                                op0=mybir.AluOpType.mult,
                                op1=mybir.AluOpType.max)
                            M = mp.tile([128, DMAXB[btag], HD + 1], f16,
                                        tag="M" + btag)
                            nc.vector.tensor_tensor(
                                out=M[:, 0:dq, :],
                                in0=G[:, g0:g0 + dq, 0:HD + 1],
                                in1=tt[:, 0:dq, :].to_broadcast(
                                    [128, dq, HD + 1]),
                                op=mybir.AluOpType.mult)
                            last_bank = (btag == "b") or db == 0
                            for d in range(dq):
                                mm = nc.tensor.matmul(
                                    out=po[:, 0:HD + 1], lhsT=ident[:],
                                    rhs=M[:, d, :],
                                    start=first, stop=last_bank and d == dq - 1)
                                if not first:
                                    mm.ldweights = False
                                first = False
                    if pend is not None:
                        stage2_block(*pend)
                    pend = (b0, b1, pos)
                stage2_block(*pend)

            def make_biases(layer):
                # c_j = exp((slope-1)*adst_j); exp(adst_j) factors out of the
                # segment softmax so only this ratio term remains per-round
                nc.scalar.activation(
                    out=cbt[layer][:], in_=adst_sb[layer][:],
                    func=mybir.ActivationFunctionType.Exp,
                    bias=0.0, scale=cfg["slope"] - 1.0)

            phase_A1()
            patch_dummy(table1)
            make_biases(0)
            phase_B(0, table1, cbt[0])
            patch_dummy(table2)
            make_biases(1)
            phase_B(1, table2, cbt[1])

    nc.compile()
    return nc


def _agch(R):
    if R < 8:
        return [0, R]
    return [0, (R * 6) // 7, R]


def _make_cfg(N, F, H):
    if N >= 32768:
        return dict(N=N, R=98, baseA=32768, baseB=67585, span=32766,
                    F=F, H=H, slope=0.2, agch=_agch(98))
    NTOT = max(2048, ((N + 128 + 1023) // 1024) * 1024)
    R = NTOT // 1024
    return dict(N=N, R=R, baseA=NTOT // 4, baseB=(3 * NTOT) // 4,
                span=min(32766, (5 * NTOT) // 8), F=F, H=H, slope=0.2,
                agch=_agch(R))


def kernel(x, edge_index, W1, a1_src, a1_dst, b1, W2, a2_src, a2_dst, b2):
    import sys
    if "/opt/trn_rl_repo" not in sys.path:
        sys.path.insert(0, "/opt/trn_rl_repo")
    from concourse import bass_utils

    x = np.asarray(x)
    cfg = _make_cfg(x.shape[0], x.shape[1], np.asarray(W1).shape[1])
    prep = _host_prep(x, edge_index, cfg)
    key = (cfg["N"], cfg["R"], prep["SA"], prep["SB"],
           tuple(prep["DA"]), tuple(prep["DB"]))
    if key not in _CACHE:
        _CACHE[key] = _build(cfg, prep["DA"], prep["DB"], prep["SA"], prep["SB"])
    nc = _CACHE[key]

    W1f = np.asarray(W1, dtype=np.float32)
    W2f = np.asarray(W2, dtype=np.float32)
    aug1 = np.concatenate(
        [W1f, np.zeros((W1f.shape[0], 1), dtype=np.float32),
         (W1f @ np.asarray(a1_src, dtype=np.float32))[:, None],
         (W1f @ np.asarray(a1_dst, dtype=np.float32))[:, None]],
        axis=1).astype(np.float16)
    aug2 = np.concatenate(
        [W2f, np.zeros((W2f.shape[0], 1), dtype=np.float32),
         (W2f @ np.asarray(a2_src, dtype=np.float32))[:, None],
         (W2f @ np.asarray(a2_dst, dtype=np.float32))[:, None]],
        axis=1).astype(np.float16)
    bb1 = np.tile(np.asarray(b1, dtype=np.float16)[None, :], (128, 1))
    bb2 = np.tile(np.asarray(b2, dtype=np.float32)[None, :], (128, 1))

    in_maps = []
    for k in range(8):
        in_maps.append({
            "xT": prep["xT"][k], "idxA": prep["idxA"][k], "idxB": prep["idxB"][k],
            "aug1": aug1, "aug2": aug2, "bb1": bb1, "bb2": bb2,
        })
    _LAST["nc"] = nc
    _LAST["in_maps"] = in_maps
    res = bass_utils.run_bass_kernel_spmd(nc, in_maps, core_ids=list(range(8)))
    shards = np.concatenate([res.results[k]["out"] for k in range(8)], axis=0)
    return shards[prep["out_pos"]].astype(np.float32)

